# revision 21
# baseline (speedup 1.0000x reference)
"""Trainium2 Bass kernel for nn_GRNNTransformGated (recursive tree GRNN over
1024 independent 10-level binary jets) — v4.

Strategy:
  - Data-parallel over jets: 8 cores x 128 trees each; whole recursion in SBUF.
  - Feature-major layout [H=128 partitions, nodes free]; natural node order
    per level (children of parent k at 2k, 2k+1).
  - Supertiles of 1024 nodes (2 matmul halves of 512); weights stationary
    across both halves (fewer effective LDWEIGHTS stalls, denser PE stream so
    the HAM clock-gate stays warm).
  - fp8(e4m3) DoubleRow matmuls for the r and z GEMMs: the (hL,hR) rhs pair is
    a [K,2,N] AP straight over the interleaved fp8 level copy (pair stride 1,
    node stride 2); the (hH,u)/(u,0) pair comes from a packed hu8 tile.
    Weights pre-scaled x4 (fp8 denormal avoidance), undone in ACT scale.
  - h_H GEMM stays bf16 (fp8 would force an extra 1x-rate DVE fp8 write of
    the 3 rh planes).
  - 3-exp softmax: gates computed as softmax(z_m - z_3) with the column
    subtraction folded into W_z / b_z on the host; kills one exp ACT, one
    z matmul group and one combine op per supertile.
  - conv_chain collapse: f^3(x) = A*relu(w*x+b) + C; for h_H the A is folded
    into the ACT scale and the +C into the z-bias colsum + a combine stt.
  - sigmoid via tanh; the 0.5 folded into ACT scales.
  - Each level kept in bf16 (combine state) + fp8 copy (matmul operand);
    leaf level double-buffered so chunk c+1 leaves overlap chunk c tail.
"""

import sys

for _p in ("/opt/trn_rl_repo", "/root/.axon_site/_ro/trn_rl_repo"):
    if _p not in sys.path:
        sys.path.insert(0, _p)

import numpy as np

B = 1024
L = 10
H = 128
FEAT = 7
NCORES = 8
TPC = B // NCORES          # trees per core = 128
TCH = 16                   # trees per chunk
NCHUNK = TPC // TCH        # 8 chunks
NPC = TPC * (2 ** L - 1)   # nodes per core = 130944
LOFF = [TPC * (2 ** j - 1) for j in range(L + 1)]
LEVEL_SIZES = [B * 2 ** j for j in range(L)]
OFF = np.concatenate([[0], np.cumsum(LEVEL_SIZES)]).astype(int)
INNER = LEVEL_SIZES[:-1]
COFF = np.concatenate([[0], np.cumsum(INNER)]).astype(int)

ST = 1024   # supertile width (nodes)
MMT = 512   # matmul half-tile

_CACHE = {}


def _children_canonical(children):
    for j in range(L - 1):
        n = INNER[j]
        blk = children[COFF[j]:COFF[j + 1]]
        base = 2 * np.arange(n, dtype=np.int64)
        if not (np.array_equal(blk[:, 0], base) and np.array_equal(blk[:, 1], base + 1)):
            return False
    return True


def _numpy_fallback(contents, children, W_u, b_u, W_h, b_h, W_z, b_z, W_r, b_r,
                    conv_w, conv_b):
    w, b = float(conv_w[0]), float(conv_b[0])

    def conv_chain(x):
        for _ in range(3):
            x = np.maximum(w * x + b, 0.0)
        return x

    def sigmoid(x):
        return 1.0 / (1.0 + np.exp(-x))

    emb = None
    for j in reversed(range(L)):
        c = contents[OFF[j]:OFF[j + 1]]
        u = conv_chain(c @ W_u + b_u)
        if j == L - 1:
            emb = u
            continue
        ch = children[COFF[j]:COFF[j + 1]]
        h_L = emb[ch[:, 0]]
        h_R = emb[ch[:, 1]]
        hhu = np.concatenate([h_L, h_R, u], axis=1)
        r = sigmoid(hhu @ W_r + b_r)
        h_H = conv_chain((r * hhu) @ W_h + b_h)
        z = np.concatenate([h_H, hhu], axis=1) @ W_z + b_z
        zs = np.stack([z[:, :H], z[:, H:2 * H], z[:, 2 * H:3 * H], z[:, 3 * H:]], axis=-1)
        zs = zs - zs.max(axis=-1, keepdims=True)
        e = np.exp(zs)
        g = e / e.sum(axis=-1, keepdims=True)
        emb = g[..., 0] * h_H + g[..., 1] * h_L + g[..., 2] * h_R + g[..., 3] * u
    return emb.reshape(B, -1).astype(np.float32)


WSCALE = 4.0  # fp8 weight pre-scale (avoid denormals); undone in ACT scale


def _build(cw, cb, A, C):
    from contextlib import ExitStack

    from concourse import bacc, mybir, tile

    f32 = mybir.dt.float32
    bf16 = mybir.dt.bfloat16
    f8 = mybir.dt.float8e4
    AF = mybir.ActivationFunctionType
    OP = mybir.AluOpType
    DR = mybir.MatmulPerfMode.DoubleRow

    nc = bacc.Bacc()

    ct_d = nc.declare_dram_parameter("ct", [FEAT, NPC], bf16, isOutput=False)
    wu_d = nc.declare_dram_parameter("wu", [FEAT, H], bf16, isOutput=False)
    wr_d = nc.declare_dram_parameter("wr8", [H, 3, 2, 2, H], f8, isOutput=False)
    wh_d = nc.declare_dram_parameter("whb", [H, 3, H], bf16, isOutput=False)
    wz_d = nc.declare_dram_parameter("wz8", [H, 3, 2, 2, H], f8, isOutput=False)
    bv_d = nc.declare_dram_parameter("bvec", [H, 8], f32, isOutput=False)
    id_d = nc.declare_dram_parameter("ident", [H, H], f32, isOutput=False)
    out_d = nc.declare_dram_parameter("out", [TPC, H], f32, isOutput=True)

    do_affine = not (A == 1.0 and C == 0.0)

    with ExitStack() as ctx:
        tc = ctx.enter_context(tile.TileContext(nc))
        wpool = ctx.enter_context(tc.tile_pool(name="wts", bufs=1))
        epool = ctx.enter_context(tc.tile_pool(name="emb", bufs=1))
        ctpool = ctx.enter_context(tc.tile_pool(name="ct", bufs=3))
        upool = ctx.enter_context(tc.tile_pool(name="utmp", bufs=3))
        xpool = ctx.enter_context(tc.tile_pool(name="x8", bufs=3))
        spool = ctx.enter_context(tc.tile_pool(name="tmp", bufs=2))
        pspool = ctx.enter_context(tc.tile_pool(name="ps", bufs=4, space="PSUM"))

        wu = wpool.tile([FEAT, H], bf16, tag="wu")
        wr = wpool.tile([H, 3, 2, 2, H], f8, tag="wr")
        wh = wpool.tile([H, 3, H], bf16, tag="wh")
        wz = wpool.tile([H, 3, 2, 2, H], f8, tag="wz")
        bv = wpool.tile([H, 8], f32, tag="bv")
        idt = wpool.tile([H, H], f32, tag="idt")
        nc.sync.dma_start(wu[:], wu_d[:])
        nc.sync.dma_start(wr[:], wr_d[:])
        nc.sync.dma_start(wh[:], wh_d[:])
        nc.sync.dma_start(wz[:], wz_d[:])
        nc.sync.dma_start(bv[:], bv_d[:])
        nc.sync.dma_start(idt[:], id_d[:])

        # bf16 emb level buffers + fp8 copies; leaf level double-buffered
        ebf, ef8 = {}, {}
        for key, ncols in (("9a", TCH * 512), ("9b", TCH * 512),
                           (8, TCH * 256), (7, TCH * 128),
                           (6, TCH * 64), (5, TPC * 32)):
            ebf[key] = epool.tile([H, ncols], bf16, name=f"e{key}b",
                                  tag=f"e{key}b")[:]
            ef8[key] = epool.tile([H, ncols], f8, name=f"e{key}f",
                                  tag=f"e{key}f")[:]
        pb_bf = {
            4: ebf["9a"][:, :2048], 3: ebf[8][:, :1024], 2: ebf[7][:, :512],
            1: ebf[6][:, :256],
        }
        pb_f8 = {
            4: ef8["9a"][:, :2048], 3: ef8[8][:, :1024], 2: ef8[7][:, :512],
            1: ef8[6][:, :256],
        }
        e0f = epool.tile([H, TPC], f32, tag="e0f")

        # prime the zero pad plane of the rotating hu8 pool
        for i in range(3):
            hu = xpool.tile([H, 3, ST], f8, name=f"hu{i}", tag="hu8")
            nc.gpsimd.memset(hu[:, 2, :], 0.0)

        def halves(n):
            return [(0, n)] if n <= MMT else [(0, MMT), (MMT, n - MMT)]

        def pair_view(buf, p0, n):
            """(hL,hR) DoubleRow rhs over interleaved child buf:
            [H, 2, n] AP, pair stride 1, node stride 2, base 2*p0."""
            return buf.rearrange("p (n two) -> p two n", two=2)[
                :, :, p0:p0 + n]

        def leaf_supertile(ct_base, parent_bf, parent_f8, p0, n):
            ctt = ctpool.tile([FEAT, ST], bf16, name="ctt", tag="ctt")
            nc.sync.dma_start(ctt[:, :n], ct_d[:, ct_base:ct_base + n])
            ps = pspool.tile([H, ST], f32, name="psu", tag="ps")
            for s, w in halves(n):
                nc.tensor.matmul(ps[:, s:s + w], wu[:], ctt[:, s:s + w],
                                 start=True, stop=True)
            dst = parent_bf[:, p0:p0 + n]
            if do_affine:
                ut = upool.tile([H, ST], bf16, name="ut", tag="ut")
                nc.scalar.activation(ut[:, :n], ps[:, :n], AF.Relu,
                                     bias=bv[:, 0:1], scale=cw)
                nc.vector.tensor_scalar(dst, ut[:, :n], A, C, OP.mult, OP.add)
            else:
                nc.scalar.activation(dst, ps[:, :n], AF.Relu,
                                     bias=bv[:, 0:1], scale=cw)
            nc.vector.tensor_copy(parent_f8[:, p0:p0 + n], dst)

        def inner_supertile(j, ct_base, child_bf, child_f8,
                            parent_bf, parent_f8, pp0, p0, n, last=False):
            cb3 = child_bf.rearrange("p (n two) -> p n two", two=2)
            hL = cb3[:, p0:p0 + n, 0]
            hR = cb3[:, p0:p0 + n, 1]
            c8 = pair_view(child_f8, p0, n)

            ctt = ctpool.tile([FEAT, ST], bf16, name="ctt", tag="ctt")
            nc.sync.dma_start(ctt[:, :n], ct_d[:, ct_base:ct_base + n])

            # ---- u ----
            psu = pspool.tile([H, ST], f32, name="psu", tag="ps")
            for s, w in halves(n):
                nc.tensor.matmul(psu[:, s:s + w], wu[:], ctt[:, s:s + w],
                                 start=True, stop=True)
            ut = upool.tile([H, ST], bf16, name="ut", tag="ut")
            nc.scalar.activation(ut[:, :n], psu[:, :n], AF.Relu,
                                 bias=bv[:, 0:1], scale=cw)
            if do_affine:
                ubf = upool.tile([H, ST], bf16, name="ubf", tag="ubf")
                nc.vector.tensor_scalar(ubf[:, :n], ut[:, :n], A, C,
                                        OP.mult, OP.add)
                u = ubf[:, :n]
            else:
                u = ut[:, :n]
            hu8 = xpool.tile([H, 3, ST], f8, name="hu8", tag="hu8")
            nc.vector.tensor_copy(hu8[:, 1, :n], u)

            # ---- r gates (tanh halves) ----
            t3 = spool.tile([H, 3, ST], bf16, name="t3", tag="t3")
            for m in range(3):
                psr = pspool.tile([H, ST], f32, name=f"psr{m}", tag="ps")
                for s, w in halves(n):
                    nc.tensor.matmul(psr[:, s:s + w], wr[:, m, 0, :, :],
                                     c8[:, :, s:s + w], start=True, stop=False,
                                     perf_mode=DR)
                for s, w in halves(n):
                    nc.tensor.matmul(psr[:, s:s + w], wr[:, m, 1, :, :],
                                     hu8[:, 1:3, s:s + w], start=False,
                                     stop=True, perf_mode=DR)
                nc.scalar.activation(t3[:, m, :n], psr[:, :n], AF.Tanh,
                                     bias=bv[:, 1 + m:2 + m],
                                     scale=0.5 / WSCALE)

            # ---- rh = (t+1) * hhu  (bf16) ----
            rh3 = spool.tile([H, 3, ST], bf16, name="rh3", tag="rh3")
            nc.vector.scalar_tensor_tensor(rh3[:, 0, :n], t3[:, 0, :n], 1.0,
                                           hL, OP.add, OP.mult)
            nc.vector.scalar_tensor_tensor(rh3[:, 1, :n], t3[:, 1, :n], 1.0,
                                           hR, OP.add, OP.mult)
            nc.vector.scalar_tensor_tensor(rh3[:, 2, :n], t3[:, 2, :n], 1.0,
                                           u, OP.add, OP.mult)

            # ---- h_H (bf16; A folded into scale; +C deferred) ----
            psh = pspool.tile([H, ST], f32, name="psh", tag="ps")
            for k in range(3):
                for s, w in halves(n):
                    nc.tensor.matmul(psh[:, s:s + w], wh[:, k, :],
                                     rh3[:, k, s:s + w], start=(k == 0),
                                     stop=(k == 2))
            hbf = upool.tile([H, ST], bf16, name="hbf", tag="hbf")
            nc.scalar.activation(hbf[:, :n], psh[:, :n], AF.Relu,
                                 bias=bv[:, 4:5], scale=A * cw * 0.5)
            nc.scalar.copy(hu8[:, 0, :n], hbf[:, :n])

            # ---- z (3-exp softmax: z_m - z_3 folded into weights) ----
            e3t = spool.tile([H, 3, ST], bf16, name="e3t", tag="e3t")
            for m in range(3):
                psz = pspool.tile([H, ST], f32, name=f"psz{m}", tag="ps")
                for s, w in halves(n):
                    nc.tensor.matmul(psz[:, s:s + w], wz[:, m, 0, :, :],
                                     c8[:, :, s:s + w], start=True, stop=False,
                                     perf_mode=DR)
                for s, w in halves(n):
                    nc.tensor.matmul(psz[:, s:s + w], wz[:, m, 1, :, :],
                                     hu8[:, 0:2, s:s + w], start=False,
                                     stop=True, perf_mode=DR)
                nc.scalar.activation(e3t[:, m, :n], psz[:, :n], AF.Exp,
                                     bias=bv[:, 5 + m:6 + m], scale=1.0 / WSCALE)
            e0, e1, e2 = e3t[:, 0, :n], e3t[:, 1, :n], e3t[:, 2, :n]

            # ---- gated combine: out = (p0+p1+p2+u) / (1+e0+e1+e2) ----
            d01 = spool.tile([H, ST], bf16, name="d01", tag="d01")
            nc.gpsimd.tensor_tensor(d01[:, :n], e0, e1, OP.add)
            d32 = spool.tile([H, ST], f32, name="d32", tag="d32")
            nc.vector.scalar_tensor_tensor(d32[:, :n], d01[:, :n], 1.0,
                                           e2, OP.add, OP.add)
            rcp = spool.tile([H, ST], f32, name="rcp", tag="rcp")
            nc.vector.reciprocal_approx_fast(rcp[:, :n], d32[:, :n])

            p0 = spool.tile([H, ST], bf16, name="p0", tag="p0")
            nc.vector.scalar_tensor_tensor(p0[:, :n], hbf[:, :n], C,
                                           e0, OP.add, OP.mult)
            p1 = spool.tile([H, ST], bf16, name="p1", tag="p1")
            nc.gpsimd.tensor_tensor(p1[:, :n], e1, hL, OP.mult)
            p2 = spool.tile([H, ST], bf16, name="p2", tag="p2")
            nc.gpsimd.tensor_tensor(p2[:, :n], e2, hR, OP.mult)
            q01 = spool.tile([H, ST], bf16, name="q01", tag="q01")
            nc.vector.tensor_tensor(q01[:, :n], p0[:, :n], p1[:, :n], OP.add)
            q2u = spool.tile([H, ST], bf16, name="q2u", tag="q2u")
            nc.vector.tensor_tensor(q2u[:, :n], p2[:, :n], u, OP.add)
            pt = spool.tile([H, ST], bf16, name="pt", tag="pt")
            nc.vector.tensor_tensor(pt[:, :n], q01[:, :n], q2u[:, :n], OP.add)

            if last:
                nc.vector.tensor_tensor(e0f[:, :n], pt[:, :n], rcp[:, :n],
                                        OP.mult)
            else:
                dst = parent_bf[:, pp0:pp0 + n]
                nc.vector.tensor_tensor(dst, pt[:, :n], rcp[:, :n], OP.mult)
                nc.vector.tensor_copy(parent_f8[:, pp0:pp0 + n], dst)

        # ================= phase A: per-chunk levels 9..5 =================
        # Leaves of chunk c+1 are emitted between L7 and L6 of chunk c so the
        # PE stays dense through the narrow chunk-tail levels.
        def emit_leaves(c):
            l9 = "9a" if c % 2 == 0 else "9b"
            nleaf = TCH * 512
            for s in range(nleaf // ST):
                leaf_supertile(LOFF[9] + c * nleaf + s * ST,
                               ebf[l9], ef8[l9], s * ST, ST)

        def emit_level(c, j):
            l9 = "9a" if c % 2 == 0 else "9b"
            nj = TCH * (2 ** j)
            ckey = l9 if j == 8 else j + 1
            if j == 5:
                pbf, pf8, pbase = ebf[5], ef8[5], c * nj
            else:
                pbf, pf8, pbase = ebf[j], ef8[j], 0
            nst = (nj + ST - 1) // ST
            for s in range(nst):
                n = min(ST, nj - s * ST)
                inner_supertile(j, LOFF[j] + c * nj + s * ST,
                                ebf[ckey], ef8[ckey],
                                pbf, pf8, pbase + s * ST, s * ST, n)

        emit_leaves(0)
        for c in range(NCHUNK):
            emit_level(c, 8)
            emit_level(c, 7)
            if c + 1 < NCHUNK:
                emit_leaves(c + 1)
            emit_level(c, 6)
            emit_level(c, 5)

        # ================= phase B: levels 4..0, all trees =================
        childs = {4: (ebf[5], ef8[5]), 3: (pb_bf[4], pb_f8[4]),
                  2: (pb_bf[3], pb_f8[3]), 1: (pb_bf[2], pb_f8[2]),
                  0: (pb_bf[1], pb_f8[1])}
        for j in range(4, -1, -1):
            nj = TPC * (2 ** j)
            cbf, cf8 = childs[j]
            nst = (nj + ST - 1) // ST
            for s in range(nst):
                n = min(ST, nj - s * ST)
                if j == 0:
                    inner_supertile(0, LOFF[0] + s * ST, cbf, cf8,
                                    None, None, 0, s * ST, n, last=True)
                else:
                    inner_supertile(j, LOFF[j] + s * ST, cbf, cf8,
                                    pb_bf[j], pb_f8[j], s * ST, s * ST, n)

        # ================= output transpose + store =================
        ptp = pspool.tile([H, ST], f32, name="ptr", tag="ps")
        nc.tensor.matmul(ptp[:, :H], e0f[:, :TPC], idt[:], is_transpose=True,
                         start=True, stop=True)
        osb = spool.tile([H, H], f32, name="osb", tag="osb")
        nc.vector.tensor_copy(osb[:], ptp[:, :H])
        nc.sync.dma_start(out_d[:], osb[:])

    nc.compile()
    if not nc.is_finalized():
        nc.finalize()
    return nc


def _prepare(inputs):
    contents = np.ascontiguousarray(np.asarray(inputs["contents"], np.float32))
    W_u = np.asarray(inputs["W_u"], np.float32)
    b_u = np.asarray(inputs["b_u"], np.float32)
    W_h = np.asarray(inputs["W_h"], np.float32)
    b_h = np.asarray(inputs["b_h"], np.float32)
    W_z = np.asarray(inputs["W_z"], np.float32)
    b_z = np.asarray(inputs["b_z"], np.float32)
    W_r = np.asarray(inputs["W_r"], np.float32)
    b_r = np.asarray(inputs["b_r"], np.float32)
    cw = float(np.asarray(inputs["conv_w"]).reshape(-1)[0])
    cb = float(np.asarray(inputs["conv_b"]).reshape(-1)[0])
    A = cw * cw
    C = cw * cb + cb

    cts = np.empty((NCORES, FEAT, NPC), np.float32)
    col = 0
    for j in range(L):
        n = TPC * 2 ** j
        blk = contents[OFF[j]:OFF[j + 1]].reshape(NCORES, n, FEAT)
        cts[:, :, col:col + n] = blk.transpose(0, 2, 1)
        col += n

    import ml_dtypes

    bf = ml_dtypes.bfloat16
    f8 = ml_dtypes.float8_e4m3fn

    def to8(x):
        return np.clip(x * WSCALE, -240.0, 240.0).astype(f8)

    # W_r [3H,3H] -> [k, m, pair, i, out]; k-blocks (hL,hR),(u,0)
    Wr = W_r.reshape(3, H, 3, H)
    wr8 = np.zeros((H, 3, 2, 2, H), np.float32)
    for m in range(3):
        wr8[:, m, 0, 0, :] = Wr[0, :, m, :]
        wr8[:, m, 0, 1, :] = Wr[1, :, m, :]
        wr8[:, m, 1, 0, :] = Wr[2, :, m, :]
    # W_z with z_3 column-block subtracted: y_m = z_m - z_3 (m=0..2)
    Wz = W_z.reshape(4, H, 4, H)   # [kblk, k, m, out]
    Wzd = Wz[:, :, 0:3, :] - Wz[:, :, 3:4, :]
    wz8 = np.zeros((H, 3, 2, 2, H), np.float32)
    for m in range(3):
        wz8[:, m, 0, 0, :] = Wzd[1, :, m, :]
        wz8[:, m, 0, 1, :] = Wzd[2, :, m, :]
        wz8[:, m, 1, 0, :] = Wzd[0, :, m, :]
        wz8[:, m, 1, 1, :] = Wzd[3, :, m, :]
    # W_h [3H,H] -> [k, kblk, out] bf16
    whb = np.ascontiguousarray(W_h.reshape(3, H, H).transpose(1, 0, 2))

    bvec = np.zeros((H, 8), np.float32)
    bvec[:, 0] = cw * b_u + cb
    bvec[:, 1:4] = 0.5 * b_r.reshape(3, H).T
    bvec[:, 4] = A * (cw * b_h + cb)
    # exp bias: (bz_m - bz_3) + C*(colsum_hH(Wz_m) - colsum_hH(Wz_3))
    cs = W_z[0:H, :].sum(axis=0).reshape(4, H)   # [m, out]
    bz4 = b_z.reshape(4, H) + C * cs
    bvec[:, 5:8] = (bz4[0:3] - bz4[3:4]).T

    common = {
        "wu": np.ascontiguousarray(W_u).astype(bf),
        "wr8": to8(wr8), "whb": whb.astype(bf), "wz8": to8(wz8),
        "bvec": bvec,
        "ident": np.eye(H, dtype=np.float32),
    }
    in_maps = [dict(common, ct=np.ascontiguousarray(cts[c]).astype(bf))
               for c in range(NCORES)]
    return in_maps, cw, cb


def kernel(**inputs):
    children = np.asarray(inputs["children"])
    cw = float(np.asarray(inputs["conv_w"]).reshape(-1)[0])
    cb = float(np.asarray(inputs["conv_b"]).reshape(-1)[0])
    collapsible = (cw >= 0.0) and (cb >= 0.0)
    if not collapsible or not _children_canonical(children):
        args = {k: np.asarray(v) for k, v in inputs.items()}
        return _numpy_fallback(**args)

    from concourse.bass_utils import run_bass_kernel_spmd

    A = cw * cw
    C = cw * cb + cb

    key = (cw, cb)
    if key not in _CACHE:
        _CACHE[key] = _build(cw, cb, A, C)
    nc = _CACHE[key]

    in_maps, _, _ = _prepare(inputs)
    res = run_bass_kernel_spmd(nc, in_maps, list(range(NCORES)))
    outs = [res.results[c]["out"] for c in range(NCORES)]
    return np.ascontiguousarray(np.concatenate(outs, axis=0).astype(np.float32))


if __name__ == "__main__":
    print("kernel_v4 module loaded")


# revision 23
# speedup vs baseline: 1.0211x; 1.0211x over previous
"""Trainium2 Bass kernel for nn_GRNNTransformGated (recursive tree GRNN over
1024 independent 10-level binary jets) — v4.

Strategy:
  - Data-parallel over jets: 8 cores x 128 trees each; whole recursion in SBUF.
  - Feature-major layout [H=128 partitions, nodes free]; natural node order
    per level (children of parent k at 2k, 2k+1).
  - Supertiles of 1024 nodes (2 matmul halves of 512); weights stationary
    across both halves (fewer effective LDWEIGHTS stalls, denser PE stream so
    the HAM clock-gate stays warm).
  - fp8(e4m3) DoubleRow matmuls for the r and z GEMMs: the (hL,hR) rhs pair is
    a [K,2,N] AP straight over the interleaved fp8 level copy (pair stride 1,
    node stride 2); the (hH,u)/(u,0) pair comes from a packed hu8 tile.
    Weights pre-scaled x4 (fp8 denormal avoidance), undone in ACT scale.
  - h_H GEMM stays bf16 (fp8 would force an extra 1x-rate DVE fp8 write of
    the 3 rh planes).
  - 3-exp softmax: gates computed as softmax(z_m - z_3) with the column
    subtraction folded into W_z / b_z on the host; kills one exp ACT, one
    z matmul group and one combine op per supertile.
  - conv_chain collapse: f^3(x) = A*relu(w*x+b) + C; for h_H the A is folded
    into the ACT scale and the +C into the z-bias colsum + a combine stt.
  - sigmoid via tanh; the 0.5 folded into ACT scales.
  - Each level kept in bf16 (combine state) + fp8 copy (matmul operand);
    leaf level double-buffered so chunk c+1 leaves overlap chunk c tail.
"""

import sys

for _p in ("/opt/trn_rl_repo", "/root/.axon_site/_ro/trn_rl_repo"):
    if _p not in sys.path:
        sys.path.insert(0, _p)

import numpy as np

B = 1024
L = 10
H = 128
FEAT = 7
NCORES = 8
TPC = B // NCORES          # trees per core = 128
TCH = 16                   # trees per chunk
NCHUNK = TPC // TCH        # 8 chunks
NPC = TPC * (2 ** L - 1)   # nodes per core = 130944
LOFF = [TPC * (2 ** j - 1) for j in range(L + 1)]
LEVEL_SIZES = [B * 2 ** j for j in range(L)]
OFF = np.concatenate([[0], np.cumsum(LEVEL_SIZES)]).astype(int)
INNER = LEVEL_SIZES[:-1]
COFF = np.concatenate([[0], np.cumsum(INNER)]).astype(int)

ST = 1024   # supertile width (nodes)
MMT = 512   # matmul half-tile

_CACHE = {}


def _children_canonical(children):
    for j in range(L - 1):
        n = INNER[j]
        blk = children[COFF[j]:COFF[j + 1]]
        base = 2 * np.arange(n, dtype=np.int64)
        if not (np.array_equal(blk[:, 0], base) and np.array_equal(blk[:, 1], base + 1)):
            return False
    return True


def _numpy_fallback(contents, children, W_u, b_u, W_h, b_h, W_z, b_z, W_r, b_r,
                    conv_w, conv_b):
    w, b = float(conv_w[0]), float(conv_b[0])

    def conv_chain(x):
        for _ in range(3):
            x = np.maximum(w * x + b, 0.0)
        return x

    def sigmoid(x):
        return 1.0 / (1.0 + np.exp(-x))

    emb = None
    for j in reversed(range(L)):
        c = contents[OFF[j]:OFF[j + 1]]
        u = conv_chain(c @ W_u + b_u)
        if j == L - 1:
            emb = u
            continue
        ch = children[COFF[j]:COFF[j + 1]]
        h_L = emb[ch[:, 0]]
        h_R = emb[ch[:, 1]]
        hhu = np.concatenate([h_L, h_R, u], axis=1)
        r = sigmoid(hhu @ W_r + b_r)
        h_H = conv_chain((r * hhu) @ W_h + b_h)
        z = np.concatenate([h_H, hhu], axis=1) @ W_z + b_z
        zs = np.stack([z[:, :H], z[:, H:2 * H], z[:, 2 * H:3 * H], z[:, 3 * H:]], axis=-1)
        zs = zs - zs.max(axis=-1, keepdims=True)
        e = np.exp(zs)
        g = e / e.sum(axis=-1, keepdims=True)
        emb = g[..., 0] * h_H + g[..., 1] * h_L + g[..., 2] * h_R + g[..., 3] * u
    return emb.reshape(B, -1).astype(np.float32)


WSCALE = 4.0  # fp8 weight pre-scale (avoid denormals); undone in ACT scale


def _build(cw, cb, A, C):
    from contextlib import ExitStack

    from concourse import bacc, mybir, tile

    f32 = mybir.dt.float32
    bf16 = mybir.dt.bfloat16
    f8 = mybir.dt.float8e4
    AF = mybir.ActivationFunctionType
    OP = mybir.AluOpType
    DR = mybir.MatmulPerfMode.DoubleRow

    nc = bacc.Bacc()

    ct_d = nc.declare_dram_parameter("ct", [FEAT, NPC], bf16, isOutput=False)
    wu_d = nc.declare_dram_parameter("wu", [FEAT, H], bf16, isOutput=False)
    wr_d = nc.declare_dram_parameter("wr8", [H, 3, 2, 2, H], f8, isOutput=False)
    wh_d = nc.declare_dram_parameter("whb", [H, 3, H], bf16, isOutput=False)
    wz_d = nc.declare_dram_parameter("wz8", [H, 3, 2, 2, H], f8, isOutput=False)
    bv_d = nc.declare_dram_parameter("bvec", [H, 8], f32, isOutput=False)
    id_d = nc.declare_dram_parameter("ident", [H, H], f32, isOutput=False)
    out_d = nc.declare_dram_parameter("out", [TPC, H], f32, isOutput=True)

    do_affine = not (A == 1.0 and C == 0.0)

    with ExitStack() as ctx:
        tc = ctx.enter_context(tile.TileContext(nc))
        wpool = ctx.enter_context(tc.tile_pool(name="wts", bufs=1))
        epool = ctx.enter_context(tc.tile_pool(name="emb", bufs=1))
        ctpool = ctx.enter_context(tc.tile_pool(name="ct", bufs=3))
        upool = ctx.enter_context(tc.tile_pool(name="utmp", bufs=3))
        xpool = ctx.enter_context(tc.tile_pool(name="x8", bufs=3))
        spool = ctx.enter_context(tc.tile_pool(name="tmp", bufs=2))
        pspool = ctx.enter_context(tc.tile_pool(name="ps", bufs=4, space="PSUM"))

        wu = wpool.tile([FEAT, H], bf16, tag="wu")
        wr = wpool.tile([H, 3, 2, 2, H], f8, tag="wr")
        wh = wpool.tile([H, 3, H], bf16, tag="wh")
        wz = wpool.tile([H, 3, 2, 2, H], f8, tag="wz")
        bv = wpool.tile([H, 8], f32, tag="bv")
        idt = wpool.tile([H, H], f32, tag="idt")
        nc.sync.dma_start(wu[:], wu_d[:])
        nc.sync.dma_start(wr[:], wr_d[:])
        nc.sync.dma_start(wh[:], wh_d[:])
        nc.sync.dma_start(wz[:], wz_d[:])
        nc.sync.dma_start(bv[:], bv_d[:])
        nc.sync.dma_start(idt[:], id_d[:])

        # bf16 emb level buffers + fp8 copies; leaf level double-buffered
        ebf, ef8 = {}, {}
        for key, ncols in (("9a", TCH * 512), ("9b", TCH * 512),
                           (8, TCH * 256), (7, TCH * 128),
                           (6, TCH * 64), (5, TPC * 32)):
            ebf[key] = epool.tile([H, ncols], bf16, name=f"e{key}b",
                                  tag=f"e{key}b")[:]
            ef8[key] = epool.tile([H, ncols], f8, name=f"e{key}f",
                                  tag=f"e{key}f")[:]
        pb_bf = {
            4: ebf["9a"][:, :2048], 3: ebf[8][:, :1024], 2: ebf[7][:, :512],
            1: ebf[6][:, :256],
        }
        pb_f8 = {
            4: ef8["9a"][:, :2048], 3: ef8[8][:, :1024], 2: ef8[7][:, :512],
            1: ef8[6][:, :256],
        }
        e0f = epool.tile([H, TPC], f32, tag="e0f")

        # prime the zero pad plane of the rotating hu8 pool
        for i in range(3):
            hu = xpool.tile([H, 3, ST], f8, name=f"hu{i}", tag="hu8")
            nc.gpsimd.memset(hu[:, 2, :], 0.0)

        def halves(n):
            return [(0, n)] if n <= MMT else [(0, MMT), (MMT, n - MMT)]

        def pair_view(buf, p0, n):
            """(hL,hR) DoubleRow rhs over interleaved child buf:
            [H, 2, n] AP, pair stride 1, node stride 2, base 2*p0."""
            return buf.rearrange("p (n two) -> p two n", two=2)[
                :, :, p0:p0 + n]

        def leaf_supertile(ct_base, parent_bf, parent_f8, p0, n):
            ctt = ctpool.tile([FEAT, ST], bf16, name="ctt", tag="ctt")
            nc.sync.dma_start(ctt[:, :n], ct_d[:, ct_base:ct_base + n])
            ps = pspool.tile([H, ST], f32, name="psu", tag="ps")
            for s, w in halves(n):
                nc.tensor.matmul(ps[:, s:s + w], wu[:], ctt[:, s:s + w],
                                 start=True, stop=True)
            dst = parent_bf[:, p0:p0 + n]
            if do_affine:
                ut = upool.tile([H, ST], bf16, name="ut", tag="ut")
                nc.scalar.activation(ut[:, :n], ps[:, :n], AF.Relu,
                                     bias=bv[:, 0:1], scale=cw)
                nc.vector.tensor_scalar(dst, ut[:, :n], A, C, OP.mult, OP.add)
            else:
                nc.scalar.activation(dst, ps[:, :n], AF.Relu,
                                     bias=bv[:, 0:1], scale=cw)
            nc.vector.tensor_copy(parent_f8[:, p0:p0 + n], dst)

        def inner_supertile(j, ct_base, child_bf, child_f8,
                            parent_bf, parent_f8, pp0, p0, n, last=False):
            cb3 = child_bf.rearrange("p (n two) -> p n two", two=2)
            hL = cb3[:, p0:p0 + n, 0]
            hR = cb3[:, p0:p0 + n, 1]
            c8 = pair_view(child_f8, p0, n)

            ctt = ctpool.tile([FEAT, ST], bf16, name="ctt", tag="ctt")
            nc.sync.dma_start(ctt[:, :n], ct_d[:, ct_base:ct_base + n])

            # ---- u ----
            psu = pspool.tile([H, ST], f32, name="psu", tag="ps")
            for s, w in halves(n):
                nc.tensor.matmul(psu[:, s:s + w], wu[:], ctt[:, s:s + w],
                                 start=True, stop=True)
            ut = upool.tile([H, ST], bf16, name="ut", tag="ut")
            nc.scalar.activation(ut[:, :n], psu[:, :n], AF.Relu,
                                 bias=bv[:, 0:1], scale=cw)
            if do_affine:
                ubf = upool.tile([H, ST], bf16, name="ubf", tag="ubf")
                nc.vector.tensor_scalar(ubf[:, :n], ut[:, :n], A, C,
                                        OP.mult, OP.add)
                u = ubf[:, :n]
            else:
                u = ut[:, :n]
            # fp8 u holds the pre-affine relu (cast runs parallel to the ts);
            # A is folded into the u-block weights, C into tanh/exp biases.
            hu8 = xpool.tile([H, 3, ST], f8, name="hu8", tag="hu8")
            nc.vector.tensor_copy(hu8[:, 1, :n], ut[:, :n])

            # ---- r gates (tanh halves) ----
            t3 = spool.tile([H, 3, ST], bf16, name="t3", tag="t3")
            for m in range(3):
                psr = pspool.tile([H, ST], f32, name=f"psr{m}", tag="ps")
                for s, w in halves(n):
                    nc.tensor.matmul(psr[:, s:s + w], wr[:, m, 0, :, :],
                                     c8[:, :, s:s + w], start=True, stop=False,
                                     perf_mode=DR)
                for s, w in halves(n):
                    nc.tensor.matmul(psr[:, s:s + w], wr[:, m, 1, :, :],
                                     hu8[:, 1:3, s:s + w], start=False,
                                     stop=True, perf_mode=DR)
                nc.scalar.activation(t3[:, m, :n], psr[:, :n], AF.Tanh,
                                     bias=bv[:, 1 + m:2 + m],
                                     scale=0.5 / WSCALE)

            # ---- rh = (t+1) * hhu  (bf16) ----
            rh3 = spool.tile([H, 3, ST], bf16, name="rh3", tag="rh3")
            nc.vector.scalar_tensor_tensor(rh3[:, 0, :n], t3[:, 0, :n], 1.0,
                                           hL, OP.add, OP.mult)
            nc.vector.scalar_tensor_tensor(rh3[:, 1, :n], t3[:, 1, :n], 1.0,
                                           hR, OP.add, OP.mult)
            nc.vector.scalar_tensor_tensor(rh3[:, 2, :n], t3[:, 2, :n], 1.0,
                                           u, OP.add, OP.mult)

            # ---- h_H (bf16; A folded into scale; +C deferred) ----
            psh = pspool.tile([H, ST], f32, name="psh", tag="ps")
            for k in range(3):
                for s, w in halves(n):
                    nc.tensor.matmul(psh[:, s:s + w], wh[:, k, :],
                                     rh3[:, k, s:s + w], start=(k == 0),
                                     stop=(k == 2))
            hbf = upool.tile([H, ST], bf16, name="hbf", tag="hbf")
            nc.scalar.activation(hbf[:, :n], psh[:, :n], AF.Relu,
                                 bias=bv[:, 4:5], scale=A * cw * 0.5)
            nc.vector.tensor_copy(hu8[:, 0, :n], hbf[:, :n])

            # ---- z (3-exp softmax: z_m - z_3 folded into weights) ----
            e3t = spool.tile([H, 3, ST], bf16, name="e3t", tag="e3t")
            for m in range(3):
                psz = pspool.tile([H, ST], f32, name=f"psz{m}", tag="ps")
                for s, w in halves(n):
                    nc.tensor.matmul(psz[:, s:s + w], wz[:, m, 0, :, :],
                                     c8[:, :, s:s + w], start=True, stop=False,
                                     perf_mode=DR)
                for s, w in halves(n):
                    nc.tensor.matmul(psz[:, s:s + w], wz[:, m, 1, :, :],
                                     hu8[:, 0:2, s:s + w], start=False,
                                     stop=True, perf_mode=DR)
                nc.scalar.activation(e3t[:, m, :n], psz[:, :n], AF.Exp,
                                     bias=bv[:, 5 + m:6 + m], scale=1.0 / WSCALE)
            e0, e1, e2 = e3t[:, 0, :n], e3t[:, 1, :n], e3t[:, 2, :n]

            # ---- gated combine: out = (p0+p1+p2+u) / (1+e0+e1+e2) ----
            d01 = spool.tile([H, ST], bf16, name="d01", tag="d01")
            nc.gpsimd.tensor_tensor(d01[:, :n], e0, e1, OP.add)
            d32 = spool.tile([H, ST], f32, name="d32", tag="d32")
            nc.vector.scalar_tensor_tensor(d32[:, :n], d01[:, :n], 1.0,
                                           e2, OP.add, OP.add)
            rcp = spool.tile([H, ST], f32, name="rcp", tag="rcp")
            nc.vector.reciprocal_approx_fast(rcp[:, :n], d32[:, :n])

            p0 = spool.tile([H, ST], bf16, name="p0", tag="p0")
            nc.vector.scalar_tensor_tensor(p0[:, :n], hbf[:, :n], C,
                                           e0, OP.add, OP.mult)
            p1 = spool.tile([H, ST], bf16, name="p1", tag="p1")
            nc.gpsimd.tensor_tensor(p1[:, :n], e1, hL, OP.mult)
            p2 = spool.tile([H, ST], bf16, name="p2", tag="p2")
            nc.gpsimd.tensor_tensor(p2[:, :n], e2, hR, OP.mult)
            q01 = spool.tile([H, ST], bf16, name="q01", tag="q01")
            nc.vector.tensor_tensor(q01[:, :n], p0[:, :n], p1[:, :n], OP.add)
            q2u = spool.tile([H, ST], bf16, name="q2u", tag="q2u")
            nc.vector.tensor_tensor(q2u[:, :n], p2[:, :n], u, OP.add)
            pt = spool.tile([H, ST], bf16, name="pt", tag="pt")
            nc.vector.tensor_tensor(pt[:, :n], q01[:, :n], q2u[:, :n], OP.add)

            if last:
                nc.vector.tensor_tensor(e0f[:, :n], pt[:, :n], rcp[:, :n],
                                        OP.mult)
            else:
                dst = parent_bf[:, pp0:pp0 + n]
                nc.vector.tensor_tensor(dst, pt[:, :n], rcp[:, :n], OP.mult)
                nc.vector.tensor_copy(parent_f8[:, pp0:pp0 + n], dst)

        # ================= phase A: per-chunk levels 9..5 =================
        # Leaves of chunk c+1 are emitted between L7 and L6 of chunk c so the
        # PE stays dense through the narrow chunk-tail levels.
        def emit_leaves(c):
            l9 = "9a" if c % 2 == 0 else "9b"
            nleaf = TCH * 512
            for s in range(nleaf // ST):
                leaf_supertile(LOFF[9] + c * nleaf + s * ST,
                               ebf[l9], ef8[l9], s * ST, ST)

        def emit_level(c, j):
            l9 = "9a" if c % 2 == 0 else "9b"
            nj = TCH * (2 ** j)
            ckey = l9 if j == 8 else j + 1
            if j == 5:
                pbf, pf8, pbase = ebf[5], ef8[5], c * nj
            else:
                pbf, pf8, pbase = ebf[j], ef8[j], 0
            nst = (nj + ST - 1) // ST
            for s in range(nst):
                n = min(ST, nj - s * ST)
                inner_supertile(j, LOFF[j] + c * nj + s * ST,
                                ebf[ckey], ef8[ckey],
                                pbf, pf8, pbase + s * ST, s * ST, n)

        emit_leaves(0)
        for c in range(NCHUNK):
            emit_level(c, 8)
            emit_level(c, 7)
            if c + 1 < NCHUNK:
                emit_leaves(c + 1)
            emit_level(c, 6)
            emit_level(c, 5)

        # ================= phase B: levels 4..0, all trees =================
        childs = {4: (ebf[5], ef8[5]), 3: (pb_bf[4], pb_f8[4]),
                  2: (pb_bf[3], pb_f8[3]), 1: (pb_bf[2], pb_f8[2]),
                  0: (pb_bf[1], pb_f8[1])}
        for j in range(4, -1, -1):
            nj = TPC * (2 ** j)
            cbf, cf8 = childs[j]
            nst = (nj + ST - 1) // ST
            for s in range(nst):
                n = min(ST, nj - s * ST)
                if j == 0:
                    inner_supertile(0, LOFF[0] + s * ST, cbf, cf8,
                                    None, None, 0, s * ST, n, last=True)
                else:
                    inner_supertile(j, LOFF[j] + s * ST, cbf, cf8,
                                    pb_bf[j], pb_f8[j], s * ST, s * ST, n)

        # ================= output transpose + store =================
        ptp = pspool.tile([H, ST], f32, name="ptr", tag="ps")
        nc.tensor.matmul(ptp[:, :H], e0f[:, :TPC], idt[:], is_transpose=True,
                         start=True, stop=True)
        osb = spool.tile([H, H], f32, name="osb", tag="osb")
        nc.vector.tensor_copy(osb[:], ptp[:, :H])
        nc.sync.dma_start(out_d[:], osb[:])

    nc.compile()
    if not nc.is_finalized():
        nc.finalize()
    return nc


def _prepare(inputs):
    contents = np.ascontiguousarray(np.asarray(inputs["contents"], np.float32))
    W_u = np.asarray(inputs["W_u"], np.float32)
    b_u = np.asarray(inputs["b_u"], np.float32)
    W_h = np.asarray(inputs["W_h"], np.float32)
    b_h = np.asarray(inputs["b_h"], np.float32)
    W_z = np.asarray(inputs["W_z"], np.float32)
    b_z = np.asarray(inputs["b_z"], np.float32)
    W_r = np.asarray(inputs["W_r"], np.float32)
    b_r = np.asarray(inputs["b_r"], np.float32)
    cw = float(np.asarray(inputs["conv_w"]).reshape(-1)[0])
    cb = float(np.asarray(inputs["conv_b"]).reshape(-1)[0])
    A = cw * cw
    C = cw * cb + cb

    cts = np.empty((NCORES, FEAT, NPC), np.float32)
    col = 0
    for j in range(L):
        n = TPC * 2 ** j
        blk = contents[OFF[j]:OFF[j + 1]].reshape(NCORES, n, FEAT)
        cts[:, :, col:col + n] = blk.transpose(0, 2, 1)
        col += n

    import ml_dtypes

    bf = ml_dtypes.bfloat16
    f8 = ml_dtypes.float8_e4m3fn

    def to8(x):
        return np.clip(x * WSCALE, -240.0, 240.0).astype(f8)

    # W_r [3H,3H] -> [k, m, pair, i, out]; k-blocks (hL,hR),(u,0); the u block
    # is pre-scaled by A since the fp8 u operand is the pre-affine relu
    Wr = W_r.reshape(3, H, 3, H)
    wr8 = np.zeros((H, 3, 2, 2, H), np.float32)
    for m in range(3):
        wr8[:, m, 0, 0, :] = Wr[0, :, m, :]
        wr8[:, m, 0, 1, :] = Wr[1, :, m, :]
        wr8[:, m, 1, 0, :] = A * Wr[2, :, m, :]
    # W_z with z_3 column-block subtracted: y_m = z_m - z_3 (m=0..2)
    Wz = W_z.reshape(4, H, 4, H)   # [kblk, k, m, out]
    Wzd = Wz[:, :, 0:3, :] - Wz[:, :, 3:4, :]
    wz8 = np.zeros((H, 3, 2, 2, H), np.float32)
    for m in range(3):
        wz8[:, m, 0, 0, :] = Wzd[1, :, m, :]
        wz8[:, m, 0, 1, :] = Wzd[2, :, m, :]
        wz8[:, m, 1, 0, :] = Wzd[0, :, m, :]
        wz8[:, m, 1, 1, :] = A * Wzd[3, :, m, :]
    # W_h [3H,H] -> [k, kblk, out] bf16
    whb = np.ascontiguousarray(W_h.reshape(3, H, H).transpose(1, 0, 2))

    bvec = np.zeros((H, 8), np.float32)
    bvec[:, 0] = cw * b_u + cb
    # tanh bias absorbs the u-block's missing +C: 0.5*C*colsum(Wr_u)
    csru = W_r[2 * H:3 * H, :].sum(axis=0).reshape(3, H)   # [m, out]
    bvec[:, 1:4] = 0.5 * (b_r.reshape(3, H) + C * csru).T
    bvec[:, 4] = A * (cw * b_h + cb)
    # exp bias: (bz_m - bz_3) + C*colsum over the hH and u block rows
    cs = (W_z[0:H, :] .sum(axis=0)
          + W_z[3 * H:4 * H, :].sum(axis=0)).reshape(4, H)   # [m, out]
    bz4 = b_z.reshape(4, H) + C * cs
    bvec[:, 5:8] = (bz4[0:3] - bz4[3:4]).T

    common = {
        "wu": np.ascontiguousarray(W_u).astype(bf),
        "wr8": to8(wr8), "whb": whb.astype(bf), "wz8": to8(wz8),
        "bvec": bvec,
        "ident": np.eye(H, dtype=np.float32),
    }
    in_maps = [dict(common, ct=np.ascontiguousarray(cts[c]).astype(bf))
               for c in range(NCORES)]
    return in_maps, cw, cb


def kernel(**inputs):
    children = np.asarray(inputs["children"])
    cw = float(np.asarray(inputs["conv_w"]).reshape(-1)[0])
    cb = float(np.asarray(inputs["conv_b"]).reshape(-1)[0])
    collapsible = (cw >= 0.0) and (cb >= 0.0)
    if not collapsible or not _children_canonical(children):
        args = {k: np.asarray(v) for k, v in inputs.items()}
        return _numpy_fallback(**args)

    from concourse.bass_utils import run_bass_kernel_spmd

    A = cw * cw
    C = cw * cb + cb

    key = (cw, cb)
    if key not in _CACHE:
        _CACHE[key] = _build(cw, cb, A, C)
    nc = _CACHE[key]

    in_maps, _, _ = _prepare(inputs)
    res = run_bass_kernel_spmd(nc, in_maps, list(range(NCORES)))
    outs = [res.results[c]["out"] for c in range(NCORES)]
    return np.ascontiguousarray(np.concatenate(outs, axis=0).astype(np.float32))


if __name__ == "__main__":
    print("kernel_v4 module loaded")
